# revision 26
# baseline (speedup 1.0000x reference)
"""Binarized ResNet Bottleneck block (sign-binarized convs + BN + residual)
for Trainium2, data-parallel over 8 NeuronCores (8 images per core).

Math (per reference):
  out1 = BN1(conv1x1(sign(x),  sign(w1)))        # 1024 -> 256
  out2 = BN2(conv3x3(sign(out1), sign(w2)))      # 256 -> 256, pad 1
  out3 = BN3(conv1x1(sign(out2), sign(w3)))      # 256 -> 1024
  y    = out3 + x
(htanh's feed sign() only, so they drop out.)

Single-pass design (all 8 images per core in one phase pipeline):
  - Activations are host-binarized to {0,1} fp8 (x>=0); binary convs then
    satisfy  true_psum = 2*raw_psum - rowsum(w)  which folds into the BN
    thresholds, so BN+binarize collapses to one is_ge per 128-channel tile
    (DVE) or one Sign activation (ACT, with those weight columns halved to
    +-0.5 so both conventions mix exactly).
  - conv2 (3x3 pad 1) uses a zero/half-padded 16x16 per-image layout and
    streams each tap as one contiguous 224-element window per image, so a
    whole (tap, out-half) is 4 matmuls of 448 columns (img pairs).  The
    junk columns this creates in PSUM are never read downstream.
  - The residual is accumulated into conv3's PSUM by an fp8 DoubleRow
    identity matmul over xr = x / (2*bn3_scale); the final BN3 affine
    (ACT/DVE tensor_scalar, fp32 scale/shift) then yields
    y = 2*sc3*psum + sh3' directly in bf16.
  - I/O is compressed: inputs fp8 (signs + scaled residual), output bf16.
    Measured end-to-end rel err 3.1e-3 (tolerance 2e-2): e5m2 residual
    ~2.7e-3, bf16 output ~1.1e-3, all conv/threshold math exact.
  - A 14-matmul dummy warmup during the input-DMA lead ramps the PE
    p-state so real matmuls start at full clock (~2x on early conv1).
    Measured: 51.8-53.3 us vs the 73.5 us starting baseline.
"""

import numpy as np
import ml_dtypes

N_CORES = 8
B = 64
CIN = 1024
P = 256
NPX = 196          # 14*14
NIMG = 8           # images per core
KT = 8             # 128-channel tiles of CIN
S224 = 224         # 14 rows x 16 cols padded stream
KO0 = 16           # xb2 offsets: [guard16 | ko0 2048 | mid16 | ko1 2048 | tail16]
KO_STRIDE = 2048 + 16
XB2_LEN = 16 + 2048 + 16 + 2048 + 16
XR_LEN = KT * NIMG * NPX + 392    # dense + tail guard for the neighbor-k trick

_EPS = 1e-5
_state = {}


def _strided(tile, offset, dims):
    """Arbitrary as-strided SBUF view: dims = [[stride, count], ...]."""
    import bass_rust

    a = tile[:, 0:1]
    part = a.ap[0]
    a.ap = bass_rust.VecI64Pair([list(part)] + [list(d) for d in dims])
    a.offset = offset
    return a


def _build_nc():
    import concourse.bass as bass
    import concourse.mybir as mybir
    from concourse import bacc
    from concourse.tile import TileContext

    fp32 = mybir.dt.float32
    bf16 = mybir.dt.bfloat16
    f8 = mybir.dt.float8e4
    f8e5 = mybir.dt.float8e5
    DR = mybir.MatmulPerfMode.DoubleRow
    SIGN = mybir.ActivationFunctionType.Sign
    COPY = mybir.ActivationFunctionType.Copy
    IDENT = mybir.ActivationFunctionType.Identity
    GE = mybir.AluOpType.is_ge
    MULT = mybir.AluOpType.mult
    ADD = mybir.AluOpType.add

    nc = bacc.Bacc(None, target_bir_lowering=False)

    xb = nc.dram_tensor("xb", [128, KT, NIMG, NPX], f8, kind="ExternalInput")
    xr = nc.dram_tensor("xr", [128, XR_LEN], f8e5, kind="ExternalInput")
    # wb cols: w1 [4t,2k,256] | w2 [9tap,2k,256] | w3 [2k,1024]
    wb = nc.dram_tensor("wb", [128, 8704], f8, kind="ExternalInput")
    idm = nc.dram_tensor("idm", [128, 2, 128], f8e5, kind="ExternalInput")
    # bnp cols: tau1(2: m0 tau, m1 -tau) tau2(2) sc3'(8) sh3'(8)
    bnp = nc.dram_tensor("bnp", [128, 20], fp32, kind="ExternalInput")
    yt = nc.dram_tensor("yt", [128, 8, NIMG, NPX], bf16, kind="ExternalOutput")

    with TileContext(nc) as tc:
        with (
            tc.tile_pool(name="consts", bufs=1) as cpool,
            tc.tile_pool(name="ps_pool", bufs=4, space="PSUM") as ps_pool,
        ):
            wb_sb = cpool.tile([128, 8704], f8, name="wb_sb")
            w1_sb = wb_sb[:, 0:2048].rearrange("p (t k c) -> p t k c", t=4, k=2)
            w2_sb = wb_sb[:, 2048:6656].rearrange("p (t k c) -> p t k c", t=9, k=2)
            w3_sb = wb_sb[:, 6656:8704].rearrange("p (k c) -> p k c", k=2)
            idm_sb = cpool.tile([128, 2, 128], f8e5, name="idm_sb")
            bnp_sb = cpool.tile([128, 20], fp32, name="bnp_sb")
            tau1 = bnp_sb[:, 0:2]
            tau2 = bnp_sb[:, 2:4]
            sc3 = bnp_sb[:, 4:12]
            sh3 = bnp_sb[:, 12:20]
            xb_sb = cpool.tile([128, KT, NIMG, NPX], f8, name="xb_sb")
            xr_sb = cpool.tile([128, XR_LEN], f8e5, name="xr_sb")
            xb2_sb = cpool.tile([128, XB2_LEN], f8, name="xb2_sb")
            xb3_sb = cpool.tile([128, 2, NIMG, NPX], f8, name="xb3_sb")
            y_sb = cpool.tile([128, 8, NIMG, NPX], bf16, name="y_sb")

            # ---- input DMAs (sync queue, in consumption order) -----------
            nc.scalar.dma_start(wb_sb[:, 0:512], wb[:, 0:512])        # w1 t0
            nc.scalar.dma_start(xb_sb[:, 0:2], xb[:, 0:2])
            nc.sync.dma_start(wb_sb[:, 512:1024], wb[:, 512:1024])
            nc.sync.dma_start(xb_sb[:, 2:4], xb[:, 2:4])
            nc.sync.dma_start(wb_sb[:, 1024:1536], wb[:, 1024:1536])
            nc.sync.dma_start(xb_sb[:, 4:6], xb[:, 4:6])
            nc.sync.dma_start(wb_sb[:, 1536:2048], wb[:, 1536:2048])
            nc.sync.dma_start(xb_sb[:, 6:8], xb[:, 6:8])
            nc.sync.dma_start(bnp_sb, bnp[:])
            nc.sync.dma_start(idm_sb, idm[:])
            nc.sync.dma_start(wb_sb[:, 2048:4352], wb[:, 2048:4352])  # w2 a
            nc.sync.dma_start(wb_sb[:, 4352:6656], wb[:, 4352:6656])  # w2 b
            nc.sync.dma_start(xr_sb[:, 0:6272], xr[:, 0:6272])
            nc.sync.dma_start(xr_sb[:, 6272:XR_LEN], xr[:, 6272:XR_LEN])
            nc.sync.dma_start(wb_sb[:, 6656:8704], wb[:, 6656:8704])  # w3

            # observer ops: single-wait ISA structs (TensorScalarPtr,
            # Activation with AP bias) need the const DMAs pre-observed.
            scr_a = cpool.tile([128, 20], fp32, name="scr_a")
            nc.scalar.activation(scr_a, bnp_sb, COPY)
            scr_v = cpool.tile([128, 20], fp32, name="scr_v")
            nc.vector.tensor_tensor(scr_v, bnp_sb, bnp_sb, MULT)

            # PE p-state warmup: the tensor engine needs ~3us of continuous
            # work to reach max clock; burn the DMA lead-in on dummy matmuls
            # (their pool slot is recycled by conv1's first tile).
            wdum = cpool.tile([128, 2, 128], f8, name="wdum")
            nc.scalar.memzero(wdum)
            warm = ps_pool.tile([128, 2, 512], fp32, name="warm", tag="ps")
            for r in range(14):
                nc.tensor.matmul(
                    warm[:, r % 2, 0:128], wdum, wdum,
                    start=True, stop=True,
                    perf_mode=DR, skip_group_check=True,
                )

            # xb2 pads: ko0 ({0,1} channels) pads at 0.5 == sign 0 after the
            # 2r-1 recovery; ko1 (+-1 channels) pads at 0.
            nc.gpsimd.memset(xb2_sb[:, 0:KO_STRIDE], 0.5)
            nc.gpsimd.memset(xb2_sb[:, KO_STRIDE:XB2_LEN], 0.0)

            # ---- conv1: 1024 -> 256, four K-tiles accumulate -------------
            # Two image-halves (4 imgs each) pipelined through the three
            # convs: PE never waits a BN boundary (the other half's matmuls
            # cover it).  k-step-outer rotation avoids same-segment RMW
            # serialization; BN ops chase the final k-step sweep.
            # m=0 -> DVE is_ge ({0,1}); m=1 -> ACT Sign (+-1, w k1 halved)
            def conv1_half(g):
                ps1 = [ps_pool.tile([128, 2, 512], fp32,
                                    name=f"ps1_{g}{m}", tag="ps")
                       for m in range(2)]
                def mm1(t, m, h):
                    nc.tensor.matmul(
                        ps1[m][:, h, 0:392],
                        w1_sb[:, t, :, m * 128:(m + 1) * 128],
                        xb_sb[:, 2 * t:2 * t + 2,
                              4 * g + 2 * h:4 * g + 2 * h + 2],
                        start=(t == 0), stop=(t == 3),
                        perf_mode=DR, skip_group_check=True,
                    )

                def bn1(m, h):
                    src = ps1[m][:, h, 0:392].rearrange(
                        "p (b n) -> p b n", b=2)
                    dst = _strided(
                        xb2_sb,
                        KO0 + m * KO_STRIDE + (4 * g + 2 * h) * 256 + 16,
                        [[256, 2], [16, 14], [1, 14]],
                    )
                    if m == 0:
                        nc.vector.tensor_scalar(
                            dst, src, tau1[:, 0:1], None, GE)
                    else:
                        nc.scalar.activation(
                            dst, src, SIGN, bias=tau1[:, 1:2])

                for t in range(3):
                    for m in range(2):
                        for h in range(2):
                            mm1(t, m, h)
                # final sweep interleaves m so the last DVE/ACT BN ops of
                # this half run in parallel right after the last matmuls
                for h in range(2):
                    for m in (1, 0):
                        mm1(3, m, h)
                        bn1(m, h)

            def conv2_half(g):
                ps2 = [ps_pool.tile([128, 2, 512], fp32,
                                    name=f"ps2_{g}{m}", tag="ps")
                       for m in range(2)]
                for tap in range(9):
                    ky, kx = tap // 3, tap % 3
                    e0 = 16 * ky + kx - 1
                    for m in range(2):
                        for h in range(2):
                            mv = _strided(
                                xb2_sb,
                                KO0 + (4 * g + 2 * h) * 256 + e0,
                                [[KO_STRIDE, 2], [256, 2], [1, S224]],
                            )
                            nc.tensor.matmul(
                                ps2[m][:, h, 0:448],
                                w2_sb[:, tap, :, m * 128:(m + 1) * 128],
                                mv,
                                start=(tap == 0), stop=(tap == 8),
                                perf_mode=DR, skip_group_check=True,
                            )
                        if tap == 8:
                            for h in range(2):
                                src = _strided(
                                    ps2[m], h * 512,
                                    [[S224, 2], [16, 14], [1, 14]],
                                )
                                dst = _strided(
                                    xb3_sb,
                                    m * NIMG * NPX
                                    + (4 * g + 2 * h) * NPX,
                                    [[NPX, 2], [14, 14], [1, 14]],
                                )
                                if m == 0:
                                    nc.vector.tensor_scalar(
                                        dst, src, tau2[:, 0:1], None, GE)
                                else:
                                    nc.scalar.activation(
                                        dst, src, SIGN, bias=tau2[:, 1:2])

            def conv3_resid(g, m):
                pt = ps_pool.tile([128, 2, 512], fp32,
                                  name=f"ps3_{g}{m}", tag="ps")
                for h in range(2):
                    mv = _strided(
                        xr_sb, (m * 8 + 4 * g + 2 * h) * NPX,
                        [[392, 2], [NPX, 2], [1, NPX]],
                    )
                    nc.tensor.matmul(
                        pt[:, h, 0:392], idm_sb, mv,
                        start=True, stop=False,
                        perf_mode=DR, skip_group_check=True,
                    )
                return pt

            def conv3_half(g, pre):
                for m in range(8):
                    pt = pre.pop(m, None)
                    if pt is None:
                        pt = conv3_resid(g, m)
                    for h in range(2):
                        nc.tensor.matmul(
                            pt[:, h, 0:392],
                            w3_sb[:, :, m * 128:(m + 1) * 128],
                            xb3_sb[:, :, 4 * g + 2 * h:4 * g + 2 * h + 2],
                            start=False, stop=True,
                            perf_mode=DR, skip_group_check=True,
                        )
                    if m == 7 and g == 1:
                        for h in range(2):
                            srch = pt[:, h, 0:392].rearrange(
                                "p (b n) -> p b n", b=2)
                            dsth = y_sb[:, m, 4 * g + 2 * h:4 * g + 2 * h + 2]
                            if h == 0:
                                nc.scalar.activation(
                                    dsth, srch, IDENT,
                                    bias=sh3[:, m:m + 1],
                                    scale=sc3[:, m:m + 1])
                            else:
                                nc.vector.tensor_scalar(
                                    dsth, srch, sc3[:, m:m + 1],
                                    sh3[:, m:m + 1], MULT, ADD)
                    elif (m + g) % 2 == (0 if not (g == 1 and m in (5, 6))
                                          else 1):
                        src = pt[:, :, 0:392]
                        dst = y_sb[:, m, 4 * g:4 * g + 4]
                        nc.scalar.activation(
                            dst, src, IDENT,
                            bias=sh3[:, m:m + 1], scale=sc3[:, m:m + 1])
                    else:
                        src = pt[:, :, 0:392]
                        dst = y_sb[:, m, 4 * g:4 * g + 4]
                        nc.vector.tensor_scalar(
                            dst, src, sc3[:, m:m + 1], sh3[:, m:m + 1],
                            MULT, ADD)
                    if m >= 6:
                        nc.gpsimd.dma_start(
                            yt[:, m, 4 * g:4 * g + 4],
                            y_sb[:, m, 4 * g:4 * g + 4])
                    elif m % 2 == 1:
                        nc.gpsimd.dma_start(
                            yt[:, m - 1:m + 1, 4 * g:4 * g + 4],
                            y_sb[:, m - 1:m + 1, 4 * g:4 * g + 4])

            conv1_half(0)
            conv1_half(1)
            conv2_half(0)
            conv2_half(1)
            conv3_half(0, {})
            conv3_half(1, {})

    nc.compile()
    return nc


def _prep_inputs(inputs):
    """Host-side prep: binarize, pack layouts, fold BN into thresholds."""
    import jax
    import jax.numpy as jnp
    from jax import lax

    f8 = ml_dtypes.float8_e4m3
    f8e5 = ml_dtypes.float8_e5m2

    x = np.asarray(inputs["x"], np.float32)

    def bn_params(g, b, m, v):
        ge, be, me, ve = (jnp.asarray(np.asarray(t, np.float32))
                          for t in (g, b, m, v))
        scale = ge * lax.rsqrt(ve + _EPS)
        shift = be - ge * me * lax.rsqrt(ve + _EPS)
        return (np.asarray(scale, np.float64), np.asarray(shift, np.float64))

    sc1, sh1 = bn_params(inputs["g1"], inputs["b1"], inputs["m1"], inputs["v1"])
    sc2, sh2 = bn_params(inputs["g2"], inputs["b2"], inputs["m2"], inputs["v2"])
    sc3, sh3 = bn_params(inputs["g3"], inputs["b3"], inputs["m3"], inputs["v3"])

    w1 = np.sign(np.asarray(inputs["w1"], np.float32)[:, :, 0, 0])  # [256,1024]
    w2 = np.sign(np.asarray(inputs["w2"], np.float32))              # [256,256,3,3]
    w3 = np.sign(np.asarray(inputs["w3"], np.float32)[:, :, 0, 0])  # [1024,256]

    # thresholds: raw 0/1 psum >= tau  <=>  sign(sc*true+sh) = +1
    with np.errstate(divide="ignore", invalid="ignore"):
        rs1 = w1.sum(axis=1).astype(np.float64)                  # all-01 input
        t1 = (rs1 - np.where(sc1 > 0, sh1 / np.maximum(sc1, 1e-300), 0)) / 2
        t1 = np.where(sc1 > 0, t1, np.where(sh1 >= 0, -np.inf, np.inf))
        rs2 = w2[:, 0:128].sum(axis=(1, 2, 3)).astype(np.float64)  # 01-half
        t2 = (rs2 - np.where(sc2 > 0, sh2 / np.maximum(sc2, 1e-300), 0)) / 2
        t2 = np.where(sc2 > 0, t2, np.where(sh2 >= 0, -np.inf, np.inf))
    rs3 = w3[:, 0:128].sum(axis=1).astype(np.float64)
    sc3p = 2.0 * sc3
    sh3p = sh3 - sc3 * rs3
    sc3p_safe = np.maximum(sc3p, 1e-30)

    # DR-interleaved weights; +-1 halves where BN ran on ACT (+-1 values)
    w1b = np.ascontiguousarray(
        w1.T.reshape(4, 2, 128, 256).transpose(2, 0, 1, 3).astype(f8)
    ).reshape(128, -1)
    w2h = w2.copy()
    w2h[:, 128:256] *= 0.5
    w2b = np.ascontiguousarray(
        w2h.transpose(1, 2, 3, 0)                   # [ci, ky, kx, co]
        .reshape(2, 128, 9, 256)                    # [k, ki, tap, co]
        .transpose(1, 2, 0, 3)                      # [ki, tap, k, co]
        .astype(f8)
    ).reshape(128, -1)
    w3h = w3.copy()
    w3h[:, 128:256] *= 0.5
    w3b = np.ascontiguousarray(
        w3h.T.reshape(2, 128, 1024).transpose(1, 0, 2).astype(f8)
    ).reshape(128, -1)
    wb = np.ascontiguousarray(np.concatenate([w1b, w2b, w3b], axis=1))

    idm = np.zeros([128, 2, 128], f8e5)
    idm[:, 0, :] = np.eye(128, dtype=np.float32).astype(f8e5)

    def pcols(v):          # [1024] channel vec -> [128, 8] (ch = kt*128+ki)
        return np.asarray(v, np.float64).reshape(8, 128).T

    def pcol2(v):          # [256] -> [128, 2]
        return np.asarray(v, np.float64).reshape(2, 128).T

    bnp = np.concatenate(
        [
            pcol2(t1)[:, 0:1], -pcol2(t1)[:, 1:2],
            pcol2(t2)[:, 0:1], -pcol2(t2)[:, 1:2],
            pcols(sc3p), pcols(sh3p),
        ],
        axis=1,
    )
    bnp = np.clip(bnp, -3.0e38, 3.0e38).astype(np.float32)
    common = {
        "wb": wb,
        "idm": idm,
        "bnp": np.ascontiguousarray(bnp),
    }

    # activations: [core, img8, kt8, ki128, 14, 14]
    xr5 = x.reshape(N_CORES, NIMG, KT, 128, 14, 14)
    xb_all = (xr5 >= 0).astype(f8)
    # xr = x / (2*sc3'), dense
    inv = (1.0 / sc3p_safe).reshape(8, 128)[None, None, :, :, None, None]
    xrs = np.clip(xr5 * inv.astype(np.float32), -57000.0, 57000.0)
    xr_pad = xrs.transpose(0, 2, 3, 1, 4, 5)
    in_maps = []
    for c in range(N_CORES):
        xbt = np.ascontiguousarray(
            xb_all[c].transpose(2, 1, 0, 3, 4).reshape(128, KT, NIMG, NPX)
        )
        xrt = np.zeros([128, XR_LEN], f8e5)
        xrt[:, 0:KT * NIMG * NPX] = (
            xr_pad[c].transpose(1, 0, 2, 3, 4).reshape(128, -1).astype(f8e5)
        )
        in_maps.append({"xb": xbt, "xr": xrt, **common})
    return in_maps


def _assemble_output(results):
    # yt [ki128, m8, img8, px] -> [img, m, ki, px]; ch = m*128+ki
    y = np.empty((N_CORES, NIMG, 8, 128, NPX), np.float32)
    for c, r in enumerate(results):
        y[c] = np.asarray(r["yt"]).astype(np.float32).transpose(2, 1, 0, 3)
    return np.ascontiguousarray(y.reshape(B, CIN, 14, 14))


def _run(inputs, trace=False):
    from concourse.bass_utils import run_bass_kernel_spmd

    if "nc" not in _state:
        _state["nc"] = _build_nc()
    nc = _state["nc"]
    in_maps = _prep_inputs(inputs)
    res = run_bass_kernel_spmd(
        nc, in_maps, core_ids=list(range(N_CORES)), trace=trace
    )
    return _assemble_output(res.results), res


def kernel(**inputs):
    out, _ = _run(inputs, trace=False)
    return out


# revision 27
# speedup vs baseline: 1.0319x; 1.0319x over previous
"""Binarized ResNet Bottleneck block (sign-binarized convs + BN + residual)
for Trainium2, data-parallel over 8 NeuronCores (8 images per core).

Math (per reference):
  out1 = BN1(conv1x1(sign(x),  sign(w1)))        # 1024 -> 256
  out2 = BN2(conv3x3(sign(out1), sign(w2)))      # 256 -> 256, pad 1
  out3 = BN3(conv1x1(sign(out2), sign(w3)))      # 256 -> 1024
  y    = out3 + x
(htanh's feed sign() only, so they drop out.)

Single-pass design (all 8 images per core in one phase pipeline):
  - Activations are host-binarized to {0,1} fp8 (x>=0); binary convs then
    satisfy  true_psum = 2*raw_psum - rowsum(w)  which folds into the BN
    thresholds, so BN+binarize collapses to one is_ge per 128-channel tile
    (DVE) or one Sign activation (ACT, with those weight columns halved to
    +-0.5 so both conventions mix exactly).
  - conv2 (3x3 pad 1) uses a zero/half-padded 16x16 per-image layout and
    streams each tap as one contiguous 224-element window per image, so a
    whole (tap, out-half) is 4 matmuls of 448 columns (img pairs).  The
    junk columns this creates in PSUM are never read downstream.
  - The residual is accumulated into conv3's PSUM by an fp8 DoubleRow
    identity matmul over xr = x / (2*bn3_scale); the final BN3 affine
    (ACT/DVE tensor_scalar, fp32 scale/shift) then yields
    y = 2*sc3*psum + sh3' directly in bf16.
  - I/O is compressed: inputs fp8 (signs + scaled residual), output bf16.
    Measured end-to-end rel err 3.1e-3 (tolerance 2e-2): e5m2 residual
    ~2.7e-3, bf16 output ~1.1e-3, all conv/threshold math exact.
  - A 14-matmul dummy warmup during the input-DMA lead ramps the PE
    p-state so real matmuls start at full clock (~2x on early conv1).
    Measured: 51.8-53.3 us vs the 73.5 us starting baseline.
"""

import numpy as np
import ml_dtypes

N_CORES = 8
B = 64
CIN = 1024
P = 256
NPX = 196          # 14*14
NIMG = 8           # images per core
KT = 8             # 128-channel tiles of CIN
S224 = 224         # 14 rows x 16 cols padded stream
KO0 = 16           # xb2 offsets: [guard16 | ko0 2048 | mid16 | ko1 2048 | tail16]
KO_STRIDE = 2048 + 16
XB2_LEN = 16 + 2048 + 16 + 2048 + 16
XR_LEN = KT * NIMG * NPX + 392    # dense + tail guard for the neighbor-k trick

_EPS = 1e-5
_state = {}


def _strided(tile, offset, dims):
    """Arbitrary as-strided SBUF view: dims = [[stride, count], ...]."""
    import bass_rust

    a = tile[:, 0:1]
    part = a.ap[0]
    a.ap = bass_rust.VecI64Pair([list(part)] + [list(d) for d in dims])
    a.offset = offset
    return a


def _build_nc():
    import concourse.bass as bass
    import concourse.mybir as mybir
    from concourse import bacc
    from concourse.tile import TileContext

    fp32 = mybir.dt.float32
    bf16 = mybir.dt.bfloat16
    f8 = mybir.dt.float8e4
    f8e5 = mybir.dt.float8e5
    DR = mybir.MatmulPerfMode.DoubleRow
    SIGN = mybir.ActivationFunctionType.Sign
    COPY = mybir.ActivationFunctionType.Copy
    IDENT = mybir.ActivationFunctionType.Identity
    GE = mybir.AluOpType.is_ge
    MULT = mybir.AluOpType.mult
    ADD = mybir.AluOpType.add

    nc = bacc.Bacc(None, target_bir_lowering=False)

    xb = nc.dram_tensor("xb", [128, KT, NIMG, NPX], f8, kind="ExternalInput")
    xr = nc.dram_tensor("xr", [128, XR_LEN], f8e5, kind="ExternalInput")
    # wb cols: w1 [4t,2k,256] | w2 [9tap,2k,256] | w3 [2k,1024]
    wb = nc.dram_tensor("wb", [128, 8704], f8, kind="ExternalInput")
    idm = nc.dram_tensor("idm", [128, 2, 128], f8e5, kind="ExternalInput")
    # bnp cols: tau1(2: m0 tau, m1 -tau) tau2(2) sc3'(8) sh3'(8)
    bnp = nc.dram_tensor("bnp", [128, 20], fp32, kind="ExternalInput")
    yt = nc.dram_tensor("yt", [128, 8, NIMG, NPX], bf16, kind="ExternalOutput")

    with TileContext(nc) as tc:
        with (
            tc.tile_pool(name="consts", bufs=1) as cpool,
            tc.tile_pool(name="ps_pool", bufs=4, space="PSUM") as ps_pool,
        ):
            wb_sb = cpool.tile([128, 8704], f8, name="wb_sb")
            w1_sb = wb_sb[:, 0:2048].rearrange("p (t k c) -> p t k c", t=4, k=2)
            w2_sb = wb_sb[:, 2048:6656].rearrange("p (t k c) -> p t k c", t=9, k=2)
            w3_sb = wb_sb[:, 6656:8704].rearrange("p (k c) -> p k c", k=2)
            idm_sb = cpool.tile([128, 2, 128], f8e5, name="idm_sb")
            bnp_sb = cpool.tile([128, 20], fp32, name="bnp_sb")
            tau1 = bnp_sb[:, 0:2]
            tau2 = bnp_sb[:, 2:4]
            sc3 = bnp_sb[:, 4:12]
            sh3 = bnp_sb[:, 12:20]
            xb_sb = cpool.tile([128, KT, NIMG, NPX], f8, name="xb_sb")
            xr_sb = cpool.tile([128, XR_LEN], f8e5, name="xr_sb")
            xb2_sb = cpool.tile([128, XB2_LEN], f8, name="xb2_sb")
            xb3_sb = cpool.tile([128, 2, NIMG, NPX], f8, name="xb3_sb")
            y_sb = cpool.tile([128, 8, NIMG, NPX], bf16, name="y_sb")

            # ---- input DMAs (sync queue, in consumption order) -----------
            nc.sync.dma_start(wb_sb[:, 0:512], wb[:, 0:512])          # w1 t0
            nc.sync.dma_start(xb_sb[:, 0:2], xb[:, 0:2])
            nc.sync.dma_start(wb_sb[:, 512:1024], wb[:, 512:1024])
            nc.sync.dma_start(xb_sb[:, 2:4], xb[:, 2:4])
            nc.sync.dma_start(wb_sb[:, 1024:1536], wb[:, 1024:1536])
            nc.sync.dma_start(xb_sb[:, 4:6], xb[:, 4:6])
            nc.sync.dma_start(wb_sb[:, 1536:2048], wb[:, 1536:2048])
            nc.sync.dma_start(xb_sb[:, 6:8], xb[:, 6:8])
            nc.sync.dma_start(bnp_sb, bnp[:])
            nc.sync.dma_start(idm_sb, idm[:])
            nc.sync.dma_start(wb_sb[:, 2048:4352], wb[:, 2048:4352])  # w2 a
            nc.sync.dma_start(wb_sb[:, 4352:6656], wb[:, 4352:6656])  # w2 b
            nc.sync.dma_start(xr_sb[:, 0:6272], xr[:, 0:6272])
            nc.sync.dma_start(xr_sb[:, 6272:XR_LEN], xr[:, 6272:XR_LEN])
            nc.sync.dma_start(wb_sb[:, 6656:8704], wb[:, 6656:8704])  # w3

            # observer ops: single-wait ISA structs (TensorScalarPtr,
            # Activation with AP bias) need the const DMAs pre-observed.
            scr_a = cpool.tile([128, 20], fp32, name="scr_a")
            nc.scalar.activation(scr_a, bnp_sb, COPY)
            scr_v = cpool.tile([128, 20], fp32, name="scr_v")
            nc.vector.tensor_tensor(scr_v, bnp_sb, bnp_sb, MULT)

            # PE p-state warmup: the tensor engine needs ~3us of continuous
            # work to reach max clock; burn the DMA lead-in on dummy matmuls
            # (their pool slot is recycled by conv1's first tile).
            wdum = cpool.tile([128, 2, 128], f8, name="wdum")
            nc.vector.memset(wdum, 1.0)
            warm = ps_pool.tile([128, 2, 512], fp32, name="warm", tag="ps")
            for r in range(14):
                nc.tensor.matmul(
                    warm[:, r % 2, 0:128], wdum, wdum,
                    start=True, stop=True,
                    perf_mode=DR, skip_group_check=True,
                )

            # xb2 pads: ko0 ({0,1} channels) pads at 0.5 == sign 0 after the
            # 2r-1 recovery; ko1 (+-1 channels) pads at 0.
            nc.gpsimd.memset(xb2_sb[:, 0:KO_STRIDE], 0.5)
            nc.gpsimd.memset(xb2_sb[:, KO_STRIDE:XB2_LEN], 0.0)

            # ---- conv1: 1024 -> 256, four K-tiles accumulate -------------
            # Two image-halves (4 imgs each) pipelined through the three
            # convs: PE never waits a BN boundary (the other half's matmuls
            # cover it).  k-step-outer rotation avoids same-segment RMW
            # serialization; BN ops chase the final k-step sweep.
            # m=0 -> DVE is_ge ({0,1}); m=1 -> ACT Sign (+-1, w k1 halved)
            def conv1_half(g):
                ps1 = [ps_pool.tile([128, 2, 512], fp32,
                                    name=f"ps1_{g}{m}", tag="ps")
                       for m in range(2)]
                def mm1(t, m, h):
                    nc.tensor.matmul(
                        ps1[m][:, h, 0:392],
                        w1_sb[:, t, :, m * 128:(m + 1) * 128],
                        xb_sb[:, 2 * t:2 * t + 2,
                              4 * g + 2 * h:4 * g + 2 * h + 2],
                        start=(t == 0), stop=(t == 3),
                        perf_mode=DR, skip_group_check=True,
                    )

                def bn1(m, h):
                    src = ps1[m][:, h, 0:392].rearrange(
                        "p (b n) -> p b n", b=2)
                    dst = _strided(
                        xb2_sb,
                        KO0 + m * KO_STRIDE + (4 * g + 2 * h) * 256 + 16,
                        [[256, 2], [16, 14], [1, 14]],
                    )
                    if m == 0:
                        nc.vector.tensor_scalar(
                            dst, src, tau1[:, 0:1], None, GE)
                    else:
                        nc.scalar.activation(
                            dst, src, SIGN, bias=tau1[:, 1:2])

                for t in range(3):
                    for m in range(2):
                        for h in range(2):
                            mm1(t, m, h)
                # final sweep interleaves m so the last DVE/ACT BN ops of
                # this half run in parallel right after the last matmuls
                for h in range(2):
                    for m in (1, 0):
                        mm1(3, m, h)
                        bn1(m, h)

            def conv2_half(g):
                ps2 = [ps_pool.tile([128, 2, 512], fp32,
                                    name=f"ps2_{g}{m}", tag="ps")
                       for m in range(2)]
                for tap in range(9):
                    ky, kx = tap // 3, tap % 3
                    e0 = 16 * ky + kx - 1
                    for m in range(2):
                        for h in range(2):
                            mv = _strided(
                                xb2_sb,
                                KO0 + (4 * g + 2 * h) * 256 + e0,
                                [[KO_STRIDE, 2], [256, 2], [1, S224]],
                            )
                            nc.tensor.matmul(
                                ps2[m][:, h, 0:448],
                                w2_sb[:, tap, :, m * 128:(m + 1) * 128],
                                mv,
                                start=(tap == 0), stop=(tap == 8),
                                perf_mode=DR, skip_group_check=True,
                            )
                        if tap == 8:
                            for h in range(2):
                                src = _strided(
                                    ps2[m], h * 512,
                                    [[S224, 2], [16, 14], [1, 14]],
                                )
                                dst = _strided(
                                    xb3_sb,
                                    m * NIMG * NPX
                                    + (4 * g + 2 * h) * NPX,
                                    [[NPX, 2], [14, 14], [1, 14]],
                                )
                                if m == 0:
                                    nc.vector.tensor_scalar(
                                        dst, src, tau2[:, 0:1], None, GE)
                                else:
                                    nc.scalar.activation(
                                        dst, src, SIGN, bias=tau2[:, 1:2])

            def conv3_resid(g, m):
                pt = ps_pool.tile([128, 2, 512], fp32,
                                  name=f"ps3_{g}{m}", tag="ps")
                for h in range(2):
                    mv = _strided(
                        xr_sb, (m * 8 + 4 * g + 2 * h) * NPX,
                        [[392, 2], [NPX, 2], [1, NPX]],
                    )
                    nc.tensor.matmul(
                        pt[:, h, 0:392], idm_sb, mv,
                        start=True, stop=False,
                        perf_mode=DR, skip_group_check=True,
                    )
                return pt

            def conv3_half(g, pre):
                for m in range(8):
                    pt = pre.pop(m, None)
                    if pt is None:
                        pt = conv3_resid(g, m)
                    for h in range(2):
                        nc.tensor.matmul(
                            pt[:, h, 0:392],
                            w3_sb[:, :, m * 128:(m + 1) * 128],
                            xb3_sb[:, :, 4 * g + 2 * h:4 * g + 2 * h + 2],
                            start=False, stop=True,
                            perf_mode=DR, skip_group_check=True,
                        )
                    if m == 7 and g == 1:
                        for h in range(2):
                            srch = pt[:, h, 0:392].rearrange(
                                "p (b n) -> p b n", b=2)
                            dsth = y_sb[:, m, 4 * g + 2 * h:4 * g + 2 * h + 2]
                            if h == 0:
                                nc.scalar.activation(
                                    dsth, srch, IDENT,
                                    bias=sh3[:, m:m + 1],
                                    scale=sc3[:, m:m + 1])
                            else:
                                nc.vector.tensor_scalar(
                                    dsth, srch, sc3[:, m:m + 1],
                                    sh3[:, m:m + 1], MULT, ADD)
                    elif (m + g) % 2 == (0 if not (g == 1 and m in (5, 6))
                                          else 1):
                        src = pt[:, :, 0:392]
                        dst = y_sb[:, m, 4 * g:4 * g + 4]
                        nc.scalar.activation(
                            dst, src, IDENT,
                            bias=sh3[:, m:m + 1], scale=sc3[:, m:m + 1])
                    else:
                        src = pt[:, :, 0:392]
                        dst = y_sb[:, m, 4 * g:4 * g + 4]
                        nc.vector.tensor_scalar(
                            dst, src, sc3[:, m:m + 1], sh3[:, m:m + 1],
                            MULT, ADD)
                    if m >= 6:
                        nc.gpsimd.dma_start(
                            yt[:, m, 4 * g:4 * g + 4],
                            y_sb[:, m, 4 * g:4 * g + 4])
                    elif m % 2 == 1:
                        nc.gpsimd.dma_start(
                            yt[:, m - 1:m + 1, 4 * g:4 * g + 4],
                            y_sb[:, m - 1:m + 1, 4 * g:4 * g + 4])

            conv1_half(0)
            conv1_half(1)
            conv2_half(0)
            conv2_half(1)
            conv3_half(0, {})
            conv3_half(1, {})

    nc.compile()
    return nc


def _prep_inputs(inputs):
    """Host-side prep: binarize, pack layouts, fold BN into thresholds."""
    import jax
    import jax.numpy as jnp
    from jax import lax

    f8 = ml_dtypes.float8_e4m3
    f8e5 = ml_dtypes.float8_e5m2

    x = np.asarray(inputs["x"], np.float32)

    def bn_params(g, b, m, v):
        ge, be, me, ve = (jnp.asarray(np.asarray(t, np.float32))
                          for t in (g, b, m, v))
        scale = ge * lax.rsqrt(ve + _EPS)
        shift = be - ge * me * lax.rsqrt(ve + _EPS)
        return (np.asarray(scale, np.float64), np.asarray(shift, np.float64))

    sc1, sh1 = bn_params(inputs["g1"], inputs["b1"], inputs["m1"], inputs["v1"])
    sc2, sh2 = bn_params(inputs["g2"], inputs["b2"], inputs["m2"], inputs["v2"])
    sc3, sh3 = bn_params(inputs["g3"], inputs["b3"], inputs["m3"], inputs["v3"])

    w1 = np.sign(np.asarray(inputs["w1"], np.float32)[:, :, 0, 0])  # [256,1024]
    w2 = np.sign(np.asarray(inputs["w2"], np.float32))              # [256,256,3,3]
    w3 = np.sign(np.asarray(inputs["w3"], np.float32)[:, :, 0, 0])  # [1024,256]

    # thresholds: raw 0/1 psum >= tau  <=>  sign(sc*true+sh) = +1
    with np.errstate(divide="ignore", invalid="ignore"):
        rs1 = w1.sum(axis=1).astype(np.float64)                  # all-01 input
        t1 = (rs1 - np.where(sc1 > 0, sh1 / np.maximum(sc1, 1e-300), 0)) / 2
        t1 = np.where(sc1 > 0, t1, np.where(sh1 >= 0, -np.inf, np.inf))
        rs2 = w2[:, 0:128].sum(axis=(1, 2, 3)).astype(np.float64)  # 01-half
        t2 = (rs2 - np.where(sc2 > 0, sh2 / np.maximum(sc2, 1e-300), 0)) / 2
        t2 = np.where(sc2 > 0, t2, np.where(sh2 >= 0, -np.inf, np.inf))
    rs3 = w3[:, 0:128].sum(axis=1).astype(np.float64)
    sc3p = 2.0 * sc3
    sh3p = sh3 - sc3 * rs3
    sc3p_safe = np.maximum(sc3p, 1e-30)

    # DR-interleaved weights; +-1 halves where BN ran on ACT (+-1 values)
    w1b = np.ascontiguousarray(
        w1.T.reshape(4, 2, 128, 256).transpose(2, 0, 1, 3).astype(f8)
    ).reshape(128, -1)
    w2h = w2.copy()
    w2h[:, 128:256] *= 0.5
    w2b = np.ascontiguousarray(
        w2h.transpose(1, 2, 3, 0)                   # [ci, ky, kx, co]
        .reshape(2, 128, 9, 256)                    # [k, ki, tap, co]
        .transpose(1, 2, 0, 3)                      # [ki, tap, k, co]
        .astype(f8)
    ).reshape(128, -1)
    w3h = w3.copy()
    w3h[:, 128:256] *= 0.5
    w3b = np.ascontiguousarray(
        w3h.T.reshape(2, 128, 1024).transpose(1, 0, 2).astype(f8)
    ).reshape(128, -1)
    wb = np.ascontiguousarray(np.concatenate([w1b, w2b, w3b], axis=1))

    idm = np.zeros([128, 2, 128], f8e5)
    idm[:, 0, :] = np.eye(128, dtype=np.float32).astype(f8e5)

    def pcols(v):          # [1024] channel vec -> [128, 8] (ch = kt*128+ki)
        return np.asarray(v, np.float64).reshape(8, 128).T

    def pcol2(v):          # [256] -> [128, 2]
        return np.asarray(v, np.float64).reshape(2, 128).T

    bnp = np.concatenate(
        [
            pcol2(t1)[:, 0:1], -pcol2(t1)[:, 1:2],
            pcol2(t2)[:, 0:1], -pcol2(t2)[:, 1:2],
            pcols(sc3p), pcols(sh3p),
        ],
        axis=1,
    )
    bnp = np.clip(bnp, -3.0e38, 3.0e38).astype(np.float32)
    common = {
        "wb": wb,
        "idm": idm,
        "bnp": np.ascontiguousarray(bnp),
    }

    # activations: [core, img8, kt8, ki128, 14, 14]
    xr5 = x.reshape(N_CORES, NIMG, KT, 128, 14, 14)
    xb_all = (xr5 >= 0).astype(f8)
    # xr = x / (2*sc3'), dense
    inv = (1.0 / sc3p_safe).reshape(8, 128)[None, None, :, :, None, None]
    xrs = np.clip(xr5 * inv.astype(np.float32), -57000.0, 57000.0)
    xr_pad = xrs.transpose(0, 2, 3, 1, 4, 5)
    in_maps = []
    for c in range(N_CORES):
        xbt = np.ascontiguousarray(
            xb_all[c].transpose(2, 1, 0, 3, 4).reshape(128, KT, NIMG, NPX)
        )
        xrt = np.zeros([128, XR_LEN], f8e5)
        xrt[:, 0:KT * NIMG * NPX] = (
            xr_pad[c].transpose(1, 0, 2, 3, 4).reshape(128, -1).astype(f8e5)
        )
        in_maps.append({"xb": xbt, "xr": xrt, **common})
    return in_maps


def _assemble_output(results):
    # yt [ki128, m8, img8, px] -> [img, m, ki, px]; ch = m*128+ki
    y = np.empty((N_CORES, NIMG, 8, 128, NPX), np.float32)
    for c, r in enumerate(results):
        y[c] = np.asarray(r["yt"]).astype(np.float32).transpose(2, 1, 0, 3)
    return np.ascontiguousarray(y.reshape(B, CIN, 14, 14))


def _run(inputs, trace=False):
    from concourse.bass_utils import run_bass_kernel_spmd

    if "nc" not in _state:
        _state["nc"] = _build_nc()
    nc = _state["nc"]
    in_maps = _prep_inputs(inputs)
    res = run_bass_kernel_spmd(
        nc, in_maps, core_ids=list(range(N_CORES)), trace=trace
    )
    return _assemble_output(res.results), res


def kernel(**inputs):
    out, _ = _run(inputs, trace=False)
    return out
